# revision 1
# baseline (speedup 1.0000x reference)
"""Trainium2 Bass kernel for ContextQueryAttention (BiDAF-style trilinear
attention). Data-parallel over batch across 8 NeuronCores (4 batches/core).

Per batch (c=1024 context rows, q=128 query rows, h=256 hidden):
  S[c,q]   = ctx@cw + (qry@qw)^T + (ctx*cqw)@qry^T + bias
  S_bar    = softmax_c(S); S_bar_bar = softmax_q(S)
  A        = S @ qry
  B        = S_bar @ (S_bar_bar^T @ ctx)
  out      = concat([ctx, A, ctx*A, ctx*B], -1)

Layout strategy: the h-contraction operands (ctx^T, qry^T) are staged
host-side during sharding (pre-rounded to fp32r), so the PE runs only true
matmuls plus 8 small exp-transposes per batch. S^T [q,c] carries s0/bias via
a K=1 augmented matmul; s1 rides the exp bias (softmax-invariant terms need
only appear where raw S is consumed). One exp pass (ACT, fused row-sums)
serves both softmaxes: softmax_q normalizers are folded into the T-matmul's
ctx operand, softmax_c normalizers into T itself, so the unnormalized exp
matrix is the lhsT of both the T and B matmuls.
"""

import numpy as np

B, C, Q, H = 32, 1024, 128, 256
N_CORES = 8
BPC = B // N_CORES  # batches per core
P = 128
HC = H // P  # h chunks of 128
CT = C // P  # c tiles of 128
CCH = 512  # S^T free-dim chunk (1 PSUM bank of fp32)
NCC = C // CCH

_NC_CACHE = {}


def _round_f32r(a):
    """Round-to-nearest f32 -> fp32r (e8m11) so the PE's fp32r path sees
    pre-rounded values (it consumes only the top 20 bits)."""
    u = a.view(np.uint32)
    return ((u + 0x800) & np.uint32(0xFFFFF000)).view(np.float32)


def _build_kernel():
    import concourse.bacc as bacc
    import concourse.bass as bass
    import concourse.tile as tile
    from concourse import mybir
    from concourse.masks import make_identity

    f32 = mybir.dt.float32
    f32r = mybir.dt.float32r
    bf16 = mybir.dt.bfloat16
    AF = mybir.ActivationFunctionType
    AX = mybir.AxisListType
    ALU = mybir.AluOpType

    nc = bacc.Bacc(trn_type="TRN2", target_bir_lowering=False, debug=False)
    ctx_d = nc.dram_tensor("ctx", [BPC, C, H], f32, kind="ExternalInput").ap()
    ctxT_d = nc.dram_tensor("ctxT", [BPC, H, C], bf16, kind="ExternalInput").ap()
    qry_d = nc.dram_tensor("qry", [BPC, Q, H], f32, kind="ExternalInput").ap()
    # host-staged in SBUF layout [p, j, b, q] so the DMA is a flat copy
    qryT_d = nc.dram_tensor("qryT", [P, HC * BPC * Q], f32, kind="ExternalInput").ap()
    cw_d = nc.dram_tensor("cw", [H], f32, kind="ExternalInput").ap()
    qw_d = nc.dram_tensor("qw", [H], f32, kind="ExternalInput").ap()
    cqw_d = nc.dram_tensor("cqw", [H], f32, kind="ExternalInput").ap()
    bias_d = nc.dram_tensor("bias", [1, 1], f32, kind="ExternalInput").ap()
    out_d = nc.dram_tensor("out", [BPC, C, 4 * H], f32, kind="ExternalOutput").ap()

    from contextlib import ExitStack

    with tile.TileContext(nc) as tc, ExitStack() as es:
        consts = es.enter_context(tc.tile_pool(name="consts", bufs=1))
        p_ctx = es.enter_context(tc.tile_pool(name="p_ctx", bufs=2))
        p_ctxT = es.enter_context(tc.tile_pool(name="p_ctxT", bufs=2))
        p_q = es.enter_context(tc.tile_pool(name="p_q", bufs=2))
        p_big = es.enter_context(tc.tile_pool(name="p_big", bufs=2))
        p_med = es.enter_context(tc.tile_pool(name="p_med", bufs=2))
        p_aug = es.enter_context(tc.tile_pool(name="p_aug", bufs=2))
        p_out = es.enter_context(tc.tile_pool(name="p_out", bufs=2))
        pp_tr = es.enter_context(tc.tile_pool(name="pp_tr", bufs=2, space="PSUM"))
        pp_st = es.enter_context(tc.tile_pool(name="pp_st", bufs=2, space="PSUM"))
        pp_mm = es.enter_context(tc.tile_pool(name="pp_mm", bufs=2, space="PSUM"))
        pp_t = es.enter_context(tc.tile_pool(name="pp_t", bufs=2, space="PSUM"))

        identity = consts.tile([P, P], bf16)
        make_identity(nc, identity)
        cw_col = consts.tile([P, HC], f32)
        nc.sync.dma_start(out=cw_col, in_=cw_d.rearrange("(j p) -> p j", p=P))
        cq_col = consts.tile([P, HC], f32)
        nc.sync.dma_start(out=cq_col, in_=cqw_d.rearrange("(j p) -> p j", p=P))
        bias_sb = consts.tile([1, 1], f32)
        nc.sync.dma_start(out=bias_sb, in_=bias_d)
        ones_c_f = consts.tile([1, C], f32)
        nc.vector.memset(ones_c_f, 1.0)
        ones_q = consts.tile([1, Q], bf16)
        nc.vector.tensor_copy(ones_q, ones_c_f[:, 0:Q])
        cw_colr = consts.tile([P, HC], bf16)
        nc.vector.tensor_copy(cw_colr, cw_col)

        ones_c = consts.tile([1, C], bf16)
        nc.vector.tensor_copy(ones_c, ones_c_f)
        qw_col = consts.tile([P, HC], f32)
        nc.sync.dma_start(out=qw_col, in_=qw_d.rearrange("(j p) -> p j", p=P))
        qw_colr = consts.tile([P, HC], bf16)
        nc.vector.tensor_copy(qw_colr, qw_col)

        # all 4 batches of query in one DMA: [q, b, h]
        q_all = consts.tile([P, BPC, H], f32)
        nc.sync.dma_start(out=q_all, in_=qry_d.rearrange("b q h -> q b h"))
        q_all_r = consts.tile([P, BPC, H], bf16)
        nc.vector.tensor_copy(q_all_r, q_all)
        # all 4 batches of qry^T in one DMA: [p, j, b, q]
        qt_all = consts.tile([P, HC, BPC, Q], f32)
        nc.sync.dma_start(out=qt_all.rearrange("p j b q -> p (j b q)"), in_=qryT_d)

        for b in range(BPC):
            qry = q_all[:, b, :]

            ctx_nat = p_ctx.tile([P, CT, H], f32, tag="ctx_nat")
            nc.sync.dma_start(
                out=ctx_nat, in_=ctx_d[b].rearrange("(t p) h -> p t h", p=P)
            )
            ctxT = p_ctxT.tile([P, HC, C], bf16, tag="ctxT")
            nc.sync.dma_start(
                out=ctxT, in_=ctxT_d[b].rearrange("(j p) c -> p j c", p=P)
            )

            # qt_cq[h, q] = qry^T * cq_weight (also the fp32r cast)
            qt_cq = p_q.tile([P, HC, Q], bf16, tag="qt_cq")
            for j in range(HC):
                nc.vector.tensor_scalar_mul(
                    qt_cq[:, j], qt_all[:, j, b, :], cq_col[:, j : j + 1]
                )

            # s1 row = (qry @ qw)^T as [1, q] (v1-style PE path)
            qt_raw = p_q.tile([P, HC, Q], bf16, tag="qt_raw")
            for j in range(HC):
                nc.vector.tensor_copy(qt_raw[:, j], qt_all[:, j, b, :])
            s1p = pp_st.tile([1, Q], f32, tag="stp")
            for j in range(HC):
                nc.tensor.matmul(
                    s1p,
                    lhsT=qw_colr[:, j : j + 1],
                    rhs=qt_raw[:, j],
                    start=(j == 0),
                    stop=(j == HC - 1),
                )
            s1_row = p_aug.tile([1, Q], bf16, tag="s1_row")
            nc.scalar.copy(s1_row, s1p)

            # ---- s0 row = ctx @ cw (+bias) as [1, c] ----
            s0_row = p_aug.tile([1, C], bf16, tag="s0_row")
            for cc in range(NCC):
                s0p = pp_st.tile([1, CCH], f32, tag="stp")
                for j in range(HC):
                    nc.tensor.matmul(
                        s0p,
                        lhsT=cw_colr[:, j : j + 1],
                        rhs=ctxT[:, j, cc * CCH : (cc + 1) * CCH],
                        start=(j == 0),
                        stop=(j == HC - 1),
                    )
                nc.scalar.activation(
                    s0_row[0:1, cc * CCH : (cc + 1) * CCH],
                    s0p,
                    AF.Identity,
                    bias=bias_sb[0:1, :],
                    scale=1.0,
                )

            # ---- S^T [q, c] = qt_cq.T @ ctxT + ones_q x s0_row; exp ----
            # (s1 rides the exp bias / the raw-S drain; it is softmax-invariant
            # along c and handled per-partition here)
            e_t = p_big.tile([P, C], bf16, tag="e_t")
            st_raw = p_big.tile([P, C], bf16, tag="st_raw")
            rsum = p_aug.tile([P, NCC], f32, tag="rsum")
            for cc in range(NCC):
                sl = slice(cc * CCH, (cc + 1) * CCH)
                stp = pp_st.tile([P, CCH], f32, tag="stp")
                for j in range(HC):
                    nc.tensor.matmul(
                        stp,
                        lhsT=qt_cq[:, j],
                        rhs=ctxT[:, j, sl],
                        start=(j == 0),
                        stop=False,
                    )
                nc.tensor.matmul(
                    stp,
                    lhsT=s1_row,
                    rhs=ones_c[:, sl],
                    start=False,
                    stop=False,
                )
                nc.tensor.matmul(
                    stp,
                    lhsT=ones_q,
                    rhs=s0_row[:, sl],
                    start=False,
                    stop=True,
                )
                nc.scalar.activation(
                    e_t[:, sl], stp, AF.Exp, accum_out=rsum[:, cc : cc + 1]
                )
                nc.vector.tensor_copy(st_raw[:, sl], stp)

            # softmax_c denominators: rq[q] = 1 / sum_c exp
            zq = p_aug.tile([P, 1], f32, tag="zq")
            nc.vector.reduce_sum(zq, rsum, axis=AX.X)
            rq = p_aug.tile([P, 1], f32, tag="rq")
            nc.vector.reciprocal(rq, zq)

            # ---- E-transpose per c-tile; softmax_q normalizers into ctx ----
            e_sb = p_med.tile([P, CT, P], bf16, tag="e_sb")
            ctx_s = p_med.tile([P, CT, H], bf16, tag="ctx_s")
            zc = p_aug.tile([P, CT], f32, tag="zc")
            rc = p_aug.tile([P, CT], f32, tag="rc")
            for t in range(CT):
                pe_ = pp_tr.tile([P, P], bf16, tag="tr")
                nc.tensor.transpose(pe_, e_t[:, t * P : (t + 1) * P], identity)
                nc.vector.reduce_sum(zc[:, t : t + 1], pe_, axis=AX.X)
                nc.vector.reciprocal(rc[:, t : t + 1], zc[:, t : t + 1])
                nc.scalar.copy(e_sb[:, t, :], pe_)
                nc.vector.tensor_scalar_mul(
                    ctx_s[:, t, :], ctx_nat[:, t, :], rc[:, t : t + 1]
                )

            # ---- T = S_bar_bar^T @ ctx as one tight accumulation group ----
            t_acc = pp_t.tile([P, H], f32, tag="t_acc")
            for t in range(CT):
                nc.tensor.matmul(
                    t_acc,
                    lhsT=e_sb[:, t, :],
                    rhs=ctx_s[:, t, :],
                    start=(t == 0),
                    stop=(t == CT - 1),
                )
            # fold softmax_c normalizer into T
            ts = p_med.tile([P, H], bf16, tag="ts")
            nc.vector.tensor_scalar_mul(ts, t_acc, rq)

            # ---- A and B per c-tile; assemble output channels ----
            out_t = p_out.tile([P, CT, 3 * H], f32, tag="out_t")
            for t in range(CT):
                sl = slice(t * P, (t + 1) * P)
                pa = pp_mm.tile([P, H], f32, tag="ab")
                nc.tensor.matmul(
                    pa,
                    lhsT=st_raw[:, sl],
                    rhs=q_all_r[:, b, :],
                    start=True,
                    stop=True,
                )
                nc.scalar.copy(out_t[:, t, 0:H], pa)
                nc.vector.tensor_mul(out_t[:, t, H : 2 * H], ctx_nat[:, t, :], pa)
                pb = pp_mm.tile([P, H], f32, tag="ab")
                nc.tensor.matmul(
                    pb,
                    lhsT=e_t[:, sl],
                    rhs=ts,
                    start=True,
                    stop=True,
                )
                nc.vector.tensor_mul(out_t[:, t, 2 * H : 3 * H], ctx_nat[:, t, :], pb)

            # ---- stores ----
            nc.sync.dma_start(
                out=out_d[b, :, 0:H].rearrange("(t p) h -> p t h", p=P), in_=ctx_nat
            )
            nc.sync.dma_start(
                out=out_d[b, :, H : 4 * H].rearrange("(t p) h -> p t h", p=P),
                in_=out_t,
            )

    nc.compile()
    return nc


def _get_nc():
    if "nc" not in _NC_CACHE:
        _NC_CACHE["nc"] = _build_kernel()
    return _NC_CACHE["nc"]


def make_in_maps(context, query, c_weight, q_weight, cq_weight, bias):
    context = np.ascontiguousarray(np.asarray(context, dtype=np.float32))
    query = np.ascontiguousarray(np.asarray(query, dtype=np.float32))
    cw = np.asarray(c_weight, dtype=np.float32).reshape(H).copy()
    qw = np.asarray(q_weight, dtype=np.float32).reshape(H).copy()
    cqw = np.asarray(cq_weight, dtype=np.float32).reshape(H).copy()
    bs = np.asarray(bias, dtype=np.float32).reshape(1, 1).copy()

    in_maps = []
    for i in range(N_CORES):
        sl = slice(i * BPC, (i + 1) * BPC)
        ctx_i = np.ascontiguousarray(context[sl])
        qry_i = np.ascontiguousarray(query[sl])
        import ml_dtypes

        ctxT_i = np.ascontiguousarray(ctx_i.transpose(0, 2, 1)).astype(ml_dtypes.bfloat16)
        # [BPC, H, Q] -> [P, HC, BPC, Q] (SBUF layout) -> flat [P, HC*BPC*Q]
        qryT_i = np.ascontiguousarray(
            qry_i.transpose(0, 2, 1)
            .reshape(BPC, HC, P, Q)
            .transpose(2, 1, 0, 3)
            .reshape(P, HC * BPC * Q)
        )
        in_maps.append(
            {
                "ctx": ctx_i,
                "ctxT": ctxT_i,
                "qry": qry_i,
                "qryT": qryT_i,
                "cw": cw,
                "qw": qw,
                "cqw": cqw,
                "bias": bs,
            }
        )
    return in_maps


def kernel(context, query, c_mask, q_mask, c_weight, q_weight, cq_weight, bias):
    from concourse import bass_utils

    nc = _get_nc()
    in_maps = make_in_maps(context, query, c_weight, q_weight, cq_weight, bias)
    res = bass_utils.run_bass_kernel_spmd(nc, in_maps, core_ids=list(range(N_CORES)))
    return np.concatenate([res.results[i]["out"] for i in range(N_CORES)], axis=0)



# revision 5
# speedup vs baseline: 1.5298x; 1.5298x over previous
"""Trainium2 Bass kernel for ContextQueryAttention (BiDAF-style trilinear
attention). Data-parallel over batch across 8 NeuronCores (4 batches/core).

Per batch (c=1024 context rows, q=128 query rows, h=256 hidden):
  S[c,q]   = ctx@cw + (qry@qw)^T + (ctx*cqw)@qry^T + bias
  S_bar    = softmax_c(S); S_bar_bar = softmax_q(S)
  A        = S @ qry
  B        = S_bar @ (S_bar_bar^T @ ctx)
  out      = concat([ctx, A, ctx*A, ctx*B], -1)

v2 layout strategy (all-bf16 I/O, minimal HBM traffic):
  - The ctx@cw term (s0[c]) is folded into the S^T contraction itself:
    lhsT[h,q] = qry^T[h,q]*cqw[h] + cw[h], so sum_h lhsT*ctxT gives
    s2^T + s0 for free.  s1+bias rides one K=1 rank-1 matmul per chunk.
  - s1 for all 4 batches comes from 2 matmuls ([1, b*q] free dim).
  - One exp pass (ACT, fused row-sums) serves both softmaxes: softmax_q
    normalizers (1/Zc) are folded into the transposed exp tiles, softmax_c
    normalizers (1/Zq) into T.  The unnormalized exp matrix is the lhsT of
    both the T and B matmuls.
  - Outputs are written bf16 (device) and upcast host-side; the ctx
    passthrough channel is assembled host-side from the f32 input.
  - A/B matmuls land in paired PSUM banks so each evacuation op covers
    two c-tiles; evac work is spread across ACT/DVE/Pool.
"""

import numpy as np

B, C, Q, H = 32, 1024, 128, 256
N_CORES = 8
BPC = B // N_CORES  # batches per core
P = 128
HC = H // P  # h chunks of 128
CT = C // P  # c tiles of 128
CCH = 512  # S^T free-dim chunk (1 PSUM bank of fp32)
NCC = C // CCH

_NC_CACHE = {}


def _build_kernel():
    import concourse.bacc as bacc
    import concourse.tile as tile
    from concourse import mybir
    from concourse.masks import make_identity

    f32 = mybir.dt.float32
    bf16 = mybir.dt.bfloat16
    AF = mybir.ActivationFunctionType
    AX = mybir.AxisListType
    ALU = mybir.AluOpType

    nc = bacc.Bacc(trn_type="TRN2", target_bir_lowering=False, debug=False)
    # host-staged layouts: per-partition contiguous for flat DMA
    ctx_d = nc.dram_tensor("ctx", [BPC, P, CT * H], bf16, kind="ExternalInput").ap()
    ctxT_d = nc.dram_tensor("ctxT", [BPC, P, HC * C], bf16, kind="ExternalInput").ap()
    qt_d = nc.dram_tensor("qt", [P, HC * BPC * Q], bf16, kind="ExternalInput").ap()
    qa_d = nc.dram_tensor("qa", [P, BPC * H], bf16, kind="ExternalInput").ap()
    wv_d = nc.dram_tensor("wv", [P, 2 * HC], f32, kind="ExternalInput").ap()
    qwr_d = nc.dram_tensor("qwr", [P, HC], bf16, kind="ExternalInput").ap()
    bias_d = nc.dram_tensor("bias", [1, 1], f32, kind="ExternalInput").ap()
    out_d = nc.dram_tensor("out", [BPC, P, CT * 3 * H], bf16, kind="ExternalOutput").ap()

    from contextlib import ExitStack

    with tile.TileContext(nc) as tc, ExitStack() as es:
        consts = es.enter_context(tc.tile_pool(name="consts", bufs=1))
        p_et = es.enter_context(tc.tile_pool(name="p_et", bufs=2))
        p_sr = es.enter_context(tc.tile_pool(name="p_sr", bufs=2))
        p_esb = es.enter_context(tc.tile_pool(name="p_esb", bufs=2))
        p_out = es.enter_context(tc.tile_pool(name="p_out", bufs=2))
        p_vec = es.enter_context(tc.tile_pool(name="p_vec", bufs=2))
        pp_st = es.enter_context(tc.tile_pool(name="pp_st", bufs=2, space="PSUM"))
        pp_tr = es.enter_context(tc.tile_pool(name="pp_tr", bufs=1, space="PSUM"))
        pp_t = es.enter_context(tc.tile_pool(name="pp_t", bufs=1, space="PSUM"))
        pp_a = es.enter_context(tc.tile_pool(name="pp_a", bufs=2, space="PSUM"))
        pp_b = es.enter_context(tc.tile_pool(name="pp_b", bufs=2, space="PSUM"))

        # ---- const DMAs (small, first so batch-0 compute starts early) ----
        wv = consts.tile([P, 2 * HC], f32)  # [:, 0:HC]=cqw cols, [:, HC:]=cw cols
        nc.sync.dma_start(out=wv, in_=wv_d)
        qwr = consts.tile([P, HC], bf16)
        nc.sync.dma_start(out=qwr, in_=qwr_d)
        bias_sb = consts.tile([1, 1], f32)
        nc.sync.dma_start(out=bias_sb, in_=bias_d)
        qt_all = consts.tile([P, HC, BPC * Q], bf16)
        nc.sync.dma_start(out=qt_all.rearrange("p j bq -> p (j bq)"), in_=qt_d)
        qa_all = consts.tile([P, BPC, H], bf16)
        nc.sync.dma_start(out=qa_all.rearrange("p b h -> p (b h)"), in_=qa_d)

        identity = consts.tile([P, P], bf16)
        make_identity(nc, identity)
        ones_c = consts.tile([1, CCH], bf16)
        nc.vector.memset(ones_c, 1.0)

        # ---- big input DMAs for first two batches ----
        ctxT_all = consts.tile([P, BPC, HC, C], bf16)
        ctx_all = consts.tile([P, BPC, CT, H], bf16)
        for b in range(2):
            nc.sync.dma_start(
                out=ctxT_all[:, b].rearrange("p j c -> p (j c)"), in_=ctxT_d[b]
            )
            nc.sync.dma_start(
                out=ctx_all[:, b].rearrange("p t h -> p (t h)"), in_=ctx_d[b]
            )

        # ---- preamble: qt_cq = qry^T*cqw + cw (folds s0), s1 rows ----
        qt_cq = consts.tile([P, HC, BPC * Q], bf16)
        for j in range(HC):
            nc.vector.tensor_scalar(
                qt_cq[:, j],
                qt_all[:, j],
                wv[:, j : j + 1],
                wv[:, HC + j : HC + j + 1],
                ALU.mult,
                ALU.add,
            )
        s1p = pp_st.tile([1, BPC * Q], f32, tag="stp")
        for j in range(HC):
            nc.tensor.matmul(
                s1p,
                lhsT=qwr[:, j : j + 1],
                rhs=qt_all[:, j],
                start=(j == 0),
                stop=(j == HC - 1),
            )
        s1_rows = consts.tile([1, BPC * Q], bf16)
        nc.scalar.activation(s1_rows, s1p, AF.Identity, bias=bias_sb[0:1, :], scale=1.0)

        for b in range(BPC):
            if b + 2 < BPC:
                nc.sync.dma_start(
                    out=ctxT_all[:, b + 2].rearrange("p j c -> p (j c)"),
                    in_=ctxT_d[b + 2],
                )
                nc.sync.dma_start(
                    out=ctx_all[:, b + 2].rearrange("p t h -> p (t h)"),
                    in_=ctx_d[b + 2],
                )

            bq = slice(b * Q, (b + 1) * Q)

            # ---- S^T [q, c] (incl s0 via qt_cq, s1+bias via rider); exp ----
            e_t = p_et.tile([P, C], bf16, tag="e_t")
            st_raw = p_sr.tile([P, C], bf16, tag="st_raw")
            rsum = p_vec.tile([P, NCC], f32, tag="rsum")
            for cc in range(NCC):
                sl = slice(cc * CCH, (cc + 1) * CCH)
                stp = pp_st.tile([P, CCH], f32, tag="stp")
                for j in range(HC):
                    nc.tensor.matmul(
                        stp,
                        lhsT=qt_cq[:, j, bq],
                        rhs=ctxT_all[:, b, j, sl],
                        start=(j == 0),
                        stop=False,
                    )
                nc.tensor.matmul(
                    stp, lhsT=s1_rows[0:1, bq], rhs=ones_c, start=False, stop=True
                )
                nc.scalar.activation(
                    e_t[:, sl], stp, AF.Exp, accum_out=rsum[:, cc : cc + 1]
                )
                nc.vector.tensor_copy(st_raw[:, sl], stp)

            # softmax_c denominators: rq[q] = 1 / sum_c exp
            zq = p_vec.tile([P, 1], f32, tag="zq")
            nc.vector.reduce_sum(zq, rsum, axis=AX.X)
            rq = p_vec.tile([P, 1], f32, tag="rq")
            nc.vector.reciprocal(rq, zq)

            # ---- E-transposes; per-tile softmax_q normalizers ----
            tr8 = pp_tr.tile([P, CT, P], bf16, tag="tr8")
            for t in range(CT):
                nc.tensor.transpose(tr8[:, t, :], e_t[:, t * P : (t + 1) * P], identity)
            e_sb = p_esb.tile([P, CT, P], bf16, tag="e_sb")
            zc8 = p_vec.tile([P, CT], f32, tag="zc8")
            rc8 = p_vec.tile([P, CT], f32, tag="rc8")
            nc.vector.reduce_sum(zc8, tr8, axis=AX.X)
            nc.vector.reciprocal(rc8, zc8)
            for t in range(CT):
                if t % 2 == 0:
                    nc.vector.tensor_scalar_mul(
                        e_sb[:, t, :], tr8[:, t, :], rc8[:, t : t + 1]
                    )
                else:
                    nc.scalar.mul(e_sb[:, t, :], tr8[:, t, :], rc8[:, t : t + 1])

            # ---- T = S_bar_bar^T @ ctx as one tight accumulation group ----
            t_acc = pp_t.tile([P, H], f32, tag="t_acc")
            for t in range(CT):
                nc.tensor.matmul(
                    t_acc,
                    lhsT=e_sb[:, t, :],
                    rhs=ctx_all[:, b, t, :],
                    start=(t == 0),
                    stop=(t == CT - 1),
                )
            ts = p_vec.tile([P, H], bf16, tag="ts")
            nc.vector.tensor_scalar_mul(ts, t_acc, rq)

            out_t = p_out.tile([P, CT, 3 * H], bf16, tag="out_t")

            # ---- B per c-tile pair; ctx*B on DVE ----
            for p2 in range(CT // 2):
                t0 = 2 * p2
                pb = pp_b.tile([P, 2, H], f32, tag="pb")
                for k in range(2):
                    nc.tensor.matmul(
                        pb[:, k, :],
                        lhsT=e_t[:, (t0 + k) * P : (t0 + k + 1) * P],
                        rhs=ts,
                        start=True,
                        stop=True,
                    )
                nc.vector.tensor_mul(
                    out_t[:, t0 : t0 + 2, 2 * H : 3 * H],
                    ctx_all[:, b, t0 : t0 + 2, :],
                    pb,
                )

            # ---- A per c-tile pair; copy on ACT, ctx*A on Pool (from SBUF A) ----
            for p2 in range(CT // 2):
                t0 = 2 * p2
                pa = pp_a.tile([P, 2, H], f32, tag="pa")
                for k in range(2):
                    nc.tensor.matmul(
                        pa[:, k, :],
                        lhsT=st_raw[:, (t0 + k) * P : (t0 + k + 1) * P],
                        rhs=qa_all[:, b, :],
                        start=True,
                        stop=True,
                    )
                nc.scalar.copy(out_t[:, t0 : t0 + 2, 0:H], pa)
                nc.gpsimd.tensor_mul(
                    out_t[:, t0 : t0 + 2, H : 2 * H],
                    ctx_all[:, b, t0 : t0 + 2, :],
                    out_t[:, t0 : t0 + 2, 0:H],
                )

            # ---- store (channels A | ctx*A | ctx*B; ctx added host-side) ----
            nc.sync.dma_start(
                out=out_d[b], in_=out_t.rearrange("p t h3 -> p (t h3)")
            )

    nc.compile()
    return nc


def _get_nc():
    if "nc" not in _NC_CACHE:
        _NC_CACHE["nc"] = _build_kernel()
    return _NC_CACHE["nc"]


def make_in_maps(context, query, c_weight, q_weight, cq_weight, bias):
    import ml_dtypes

    bf16 = ml_dtypes.bfloat16
    context = np.ascontiguousarray(np.asarray(context, dtype=np.float32))
    query = np.asarray(query, dtype=np.float32)
    cw = np.asarray(c_weight, dtype=np.float32).reshape(H)
    qw = np.asarray(q_weight, dtype=np.float32).reshape(H)
    cqw = np.asarray(cq_weight, dtype=np.float32).reshape(H)
    bs = np.asarray(bias, dtype=np.float32).reshape(1, 1).copy()

    # [:, 0:HC] = cqw cols, [:, HC:] = cw cols  (column j holds h=j*128+p)
    wv = np.concatenate(
        [cqw.reshape(HC, P).T, cw.reshape(HC, P).T], axis=1
    ).astype(np.float32)
    wv = np.ascontiguousarray(wv)
    qwr = np.ascontiguousarray(qw.reshape(HC, P).T.astype(bf16))

    in_maps = []
    for i in range(N_CORES):
        sl = slice(i * BPC, (i + 1) * BPC)
        ctx_i = context[sl]
        qry_i = query[sl]
        # ctx: [b, c, h] -> [b, p, t, h] with c = t*128+p
        ctx_s = np.ascontiguousarray(
            ctx_i.reshape(BPC, CT, P, H).transpose(0, 2, 1, 3).reshape(BPC, P, CT * H)
        ).astype(bf16)
        # ctxT: [b, h, c] -> [b, p, j, c] with h = j*128+p
        ctxT_s = np.ascontiguousarray(
            ctx_i.transpose(0, 2, 1)
            .reshape(BPC, HC, P, C)
            .transpose(0, 2, 1, 3)
            .reshape(BPC, P, HC * C)
        ).astype(bf16)
        # qry^T: [b, h, q] -> [p, j, b, q]
        qt_s = np.ascontiguousarray(
            qry_i.transpose(0, 2, 1)
            .reshape(BPC, HC, P, Q)
            .transpose(2, 1, 0, 3)
            .reshape(P, HC * BPC * Q)
        ).astype(bf16)
        # qry: [b, q, h] -> [q, b, h]
        qa_s = np.ascontiguousarray(
            qry_i.transpose(1, 0, 2).reshape(P, BPC * H)
        ).astype(bf16)
        in_maps.append(
            {
                "ctx": ctx_s,
                "ctxT": ctxT_s,
                "qt": qt_s,
                "qa": qa_s,
                "wv": wv,
                "qwr": qwr,
                "bias": bs,
            }
        )
    return in_maps


def kernel(context, query, c_mask, q_mask, c_weight, q_weight, cq_weight, bias):
    from concourse import bass_utils

    nc = _get_nc()
    in_maps = make_in_maps(context, query, c_weight, q_weight, cq_weight, bias)
    res = bass_utils.run_bass_kernel_spmd(nc, in_maps, core_ids=list(range(N_CORES)))

    context = np.asarray(context, dtype=np.float32)
    full = np.empty((B, C, 4 * H), dtype=np.float32)
    full[:, :, 0:H] = context
    for i in range(N_CORES):
        # device out: [b, p, t, 3h] -> [b, (t p), 3h]
        o = res.results[i]["out"].reshape(BPC, P, CT, 3 * H).transpose(0, 2, 1, 3)
        full[i * BPC : (i + 1) * BPC, :, H:] = o.reshape(BPC, C, 3 * H).astype(
            np.float32
        )
    return full


# revision 6
# speedup vs baseline: 1.6166x; 1.0568x over previous
"""Trainium2 Bass kernel for ContextQueryAttention (BiDAF-style trilinear
attention). Data-parallel over batch across 8 NeuronCores (4 batches/core).

Per batch (c=1024 context rows, q=128 query rows, h=256 hidden):
  S[c,q]   = ctx@cw + (qry@qw)^T + (ctx*cqw)@qry^T + bias
  S_bar    = softmax_c(S); S_bar_bar = softmax_q(S)
  A        = S @ qry
  B        = S_bar @ (S_bar_bar^T @ ctx)
  out      = concat([ctx, A, ctx*A, ctx*B], -1)

v3 layout strategy (all-bf16 I/O, minimal HBM traffic, latency-hiding):
  - ctx@cw (s0) folds into the S^T contraction (lhsT = qry^T*cqw + cw);
    s1+bias rides one K=1 rank-1 matmul per 512-col chunk.
  - One exp pass serves both softmaxes: 1/Zc folds into the transposed
    exp tiles, 1/Zq into T.  PE order S^T, tr, A, T, B keeps the PE busy
    while the zc/rc/e-scale chain runs on DVE/ACT.
  - Outputs written bf16; ctx passthrough channel assembled host-side.
  - ctx*A computed on Pool from the SBUF A-channel copy (Pool cannot
    read PSUM); evac work split ACT/DVE/Pool; consts packed in 2 DMAs.
"""

import numpy as np

B, C, Q, H = 32, 1024, 128, 256
N_CORES = 8
BPC = B // N_CORES  # batches per core
P = 128
HC = H // P  # h chunks of 128
CT = C // P  # c tiles of 128
CCH = 512  # S^T free-dim chunk (1 PSUM bank of fp32)
NCC = C // CCH

_NC_CACHE = {}


def _build_kernel():
    import concourse.bacc as bacc
    import concourse.tile as tile
    from concourse import mybir
    from concourse.masks import make_identity

    f32 = mybir.dt.float32
    bf16 = mybir.dt.bfloat16
    AF = mybir.ActivationFunctionType
    AX = mybir.AxisListType
    ALU = mybir.AluOpType

    nc = bacc.Bacc(trn_type="TRN2", target_bir_lowering=False, debug=False)
    # host-staged layouts: per-partition contiguous for flat DMA
    ctx_d = nc.dram_tensor("ctx", [BPC, P, CT * H], bf16, kind="ExternalInput").ap()
    ctxT_d = nc.dram_tensor("ctxT", [BPC, P, HC * C], bf16, kind="ExternalInput").ap()
    # packed bf16 consts: [0:1024]=qry^T, [1024:2048]=qry, [2048:2050]=qw cols
    qb_d = nc.dram_tensor("qb", [P, 2 * HC * BPC * Q + HC], bf16, kind="ExternalInput").ap()
    # packed f32 consts: [0:HC]=cqw cols, [HC:2HC]=cw cols, [2HC]=bias
    wv_d = nc.dram_tensor("wv", [P, 2 * HC + 1], f32, kind="ExternalInput").ap()
    out_d = nc.dram_tensor("out", [BPC, P, CT * 3 * H], bf16, kind="ExternalOutput").ap()

    from contextlib import ExitStack

    with tile.TileContext(nc) as tc, ExitStack() as es:
        consts = es.enter_context(tc.tile_pool(name="consts", bufs=1))
        p_et = es.enter_context(tc.tile_pool(name="p_et", bufs=2))
        p_sr = es.enter_context(tc.tile_pool(name="p_sr", bufs=2))
        p_esb = es.enter_context(tc.tile_pool(name="p_esb", bufs=2))
        p_out = es.enter_context(tc.tile_pool(name="p_out", bufs=2))
        p_vec = es.enter_context(tc.tile_pool(name="p_vec", bufs=2))
        pp_st = es.enter_context(tc.tile_pool(name="pp_st", bufs=2, space="PSUM"))
        pp_tr = es.enter_context(tc.tile_pool(name="pp_tr", bufs=1, space="PSUM"))
        pp_t = es.enter_context(tc.tile_pool(name="pp_t", bufs=1, space="PSUM"))
        pp_ab = es.enter_context(tc.tile_pool(name="pp_ab", bufs=4, space="PSUM"))

        # ---- const DMAs (2 packed transfers) ----
        wv = consts.tile([P, 2 * HC + 1], f32)
        nc.sync.dma_start(out=wv, in_=wv_d)
        qb = consts.tile([P, 2 * HC * BPC * Q + HC], bf16)
        nc.sync.dma_start(out=qb, in_=qb_d)
        qt_all = qb[:, 0 : HC * BPC * Q].rearrange("p (j bq) -> p j bq", j=HC)
        qa_all = qb[:, HC * BPC * Q : 2 * HC * BPC * Q].rearrange(
            "p (b h) -> p b h", b=BPC
        )
        qwr = qb[:, 2 * HC * BPC * Q :]
        bias_sb = wv[0:1, 2 * HC : 2 * HC + 1]

        identity = consts.tile([P, P], bf16)
        make_identity(nc, identity)
        ones_c = consts.tile([1, CCH], bf16)
        nc.vector.memset(ones_c, 1.0)

        # ---- big input DMAs for first two batches ----
        ctxT_all = consts.tile([P, BPC, HC, C], bf16)
        ctx_all = consts.tile([P, BPC, CT, H], bf16)
        for b in range(2):
            nc.sync.dma_start(
                out=ctxT_all[:, b].rearrange("p j c -> p (j c)"), in_=ctxT_d[b]
            )
            nc.sync.dma_start(
                out=ctx_all[:, b].rearrange("p t h -> p (t h)"), in_=ctx_d[b]
            )

        # ---- preamble: qt_cq = qry^T*cqw + cw (folds s0) on Pool; s1 rows ----
        qt_cq = consts.tile([P, HC, BPC * Q], bf16)
        for j in range(HC):
            nc.gpsimd.tensor_scalar(
                qt_cq[:, j],
                qt_all[:, j],
                wv[:, j : j + 1],
                wv[:, HC + j : HC + j + 1],
                ALU.mult,
                ALU.add,
            )
        s1p = pp_st.tile([1, BPC * Q], f32, tag="stp")
        for j in range(HC):
            nc.tensor.matmul(
                s1p,
                lhsT=qwr[:, j : j + 1],
                rhs=qt_all[:, j],
                start=(j == 0),
                stop=(j == HC - 1),
            )
        s1_rows = consts.tile([1, BPC * Q], bf16)
        nc.scalar.activation(s1_rows, s1p, AF.Identity, bias=bias_sb, scale=1.0)

        for b in range(BPC):
            if b + 2 < BPC:
                nc.sync.dma_start(
                    out=ctxT_all[:, b + 2].rearrange("p j c -> p (j c)"),
                    in_=ctxT_d[b + 2],
                )
                nc.sync.dma_start(
                    out=ctx_all[:, b + 2].rearrange("p t h -> p (t h)"),
                    in_=ctx_d[b + 2],
                )

            bq = slice(b * Q, (b + 1) * Q)

            # ---- S^T [q, c] (incl s0 via qt_cq, s1+bias via rider); exp ----
            e_t = p_et.tile([P, C], bf16, tag="e_t")
            st_raw = p_sr.tile([P, C], bf16, tag="st_raw")
            stps = []
            for cc in range(NCC):
                sl = slice(cc * CCH, (cc + 1) * CCH)
                stp = pp_st.tile([P, CCH], f32, tag="stp")
                for j in range(HC):
                    nc.tensor.matmul(
                        stp,
                        lhsT=qt_cq[:, j, bq],
                        rhs=ctxT_all[:, b, j, sl],
                        start=(j == 0),
                        stop=False,
                    )
                nc.tensor.matmul(
                    stp, lhsT=s1_rows[0:1, bq], rhs=ones_c, start=False, stop=True
                )
                nc.scalar.activation(e_t[:, sl], stp, AF.Exp)
                stps.append(stp)
            # raw S^T to SBUF for the A matmuls (ACT c0, DVE c1)
            nc.scalar.copy(st_raw[:, 0:CCH], stps[0])
            nc.vector.tensor_copy(st_raw[:, CCH:C], stps[1])

            # ---- E-transposes ----
            tr8 = pp_tr.tile([P, CT, P], bf16, tag="tr8")
            for t in range(CT):
                nc.tensor.transpose(tr8[:, t, :], e_t[:, t * P : (t + 1) * P], identity)

            # per-tile softmax_q normalizers, first half; e-scale t=0..3
            e_sb = p_esb.tile([P, CT, P], bf16, tag="e_sb")
            zc8 = p_vec.tile([P, CT], f32, tag="zc8")
            rc8 = p_vec.tile([P, CT], f32, tag="rc8")
            h1 = CT // 2
            nc.vector.reduce_sum(zc8[:, 0:h1], tr8[:, 0:h1, :], axis=AX.X)
            nc.vector.reciprocal(rc8[:, 0:h1], zc8[:, 0:h1])
            for t in range(h1):
                if t % 2 == 0:
                    nc.vector.tensor_scalar_mul(
                        e_sb[:, t, :], tr8[:, t, :], rc8[:, t : t + 1]
                    )
                else:
                    nc.scalar.mul(e_sb[:, t, :], tr8[:, t, :], rc8[:, t : t + 1])

            out_t = p_out.tile([P, CT, 3 * H], bf16, tag="out_t")

            # ---- A per c-tile pair (fills the PE gap while es chain runs);
            # copy on ACT, ctx*A on Pool from the SBUF A-channel ----
            for p2 in range(CT // 2):
                t0 = 2 * p2
                pa = pp_ab.tile([P, 2, H], f32, tag="ab")
                for k in range(2):
                    nc.tensor.matmul(
                        pa[:, k, :],
                        lhsT=st_raw[:, (t0 + k) * P : (t0 + k + 1) * P],
                        rhs=qa_all[:, b, :],
                        start=True,
                        stop=True,
                    )
                nc.scalar.copy(out_t[:, t0 : t0 + 2, 0:H], pa)
                nc.gpsimd.tensor_mul(
                    out_t[:, t0 : t0 + 2, H : 2 * H],
                    ctx_all[:, b, t0 : t0 + 2, :],
                    out_t[:, t0 : t0 + 2, 0:H],
                )

            # normalizers second half; e-scale t=4..7 (all DVE)
            nc.vector.reduce_sum(zc8[:, h1:CT], tr8[:, h1:CT, :], axis=AX.X)
            nc.vector.reciprocal(rc8[:, h1:CT], zc8[:, h1:CT])
            for t in range(h1, CT):
                nc.vector.tensor_scalar_mul(
                    e_sb[:, t, :], tr8[:, t, :], rc8[:, t : t + 1]
                )

            # ---- T = S_bar_bar^T @ ctx as one tight accumulation group ----
            t_acc = pp_t.tile([P, H], f32, tag="t_acc")
            for t in range(CT):
                nc.tensor.matmul(
                    t_acc,
                    lhsT=e_sb[:, t, :],
                    rhs=ctx_all[:, b, t, :],
                    start=(t == 0),
                    stop=(t == CT - 1),
                )
            # softmax_c denominators direct from e_t; fold into T
            zq = p_vec.tile([P, 1], f32, tag="zq")
            nc.vector.reduce_sum(zq, e_t, axis=AX.X)
            rq = p_vec.tile([P, 1], f32, tag="rq")
            nc.vector.reciprocal(rq, zq)
            ts = p_vec.tile([P, H], bf16, tag="ts")
            nc.vector.tensor_scalar_mul(ts, t_acc, rq)

            # ---- B per c-tile pair; ctx*B on DVE ----
            for p2 in range(CT // 2):
                t0 = 2 * p2
                pb = pp_ab.tile([P, 2, H], f32, tag="ab")
                for k in range(2):
                    nc.tensor.matmul(
                        pb[:, k, :],
                        lhsT=e_t[:, (t0 + k) * P : (t0 + k + 1) * P],
                        rhs=ts,
                        start=True,
                        stop=True,
                    )
                nc.vector.tensor_mul(
                    out_t[:, t0 : t0 + 2, 2 * H : 3 * H],
                    ctx_all[:, b, t0 : t0 + 2, :],
                    pb,
                )

            # ---- stores in halves so the first can drain early ----
            nc.sync.dma_start(
                out=out_d[b, :, 0 : 4 * 3 * H],
                in_=out_t[:, 0:4, :].rearrange("p t h3 -> p (t h3)"),
            )
            nc.sync.dma_start(
                out=out_d[b, :, 4 * 3 * H :],
                in_=out_t[:, 4:CT, :].rearrange("p t h3 -> p (t h3)"),
            )

    nc.compile()
    return nc


def _get_nc():
    if "nc" not in _NC_CACHE:
        _NC_CACHE["nc"] = _build_kernel()
    return _NC_CACHE["nc"]


def make_in_maps(context, query, c_weight, q_weight, cq_weight, bias):
    import ml_dtypes

    bf16 = ml_dtypes.bfloat16
    context = np.ascontiguousarray(np.asarray(context, dtype=np.float32))
    query = np.asarray(query, dtype=np.float32)
    cw = np.asarray(c_weight, dtype=np.float32).reshape(H)
    qw = np.asarray(q_weight, dtype=np.float32).reshape(H)
    cqw = np.asarray(cq_weight, dtype=np.float32).reshape(H)
    bs = float(np.asarray(bias, dtype=np.float32).reshape(1)[0])

    # [:, 0:HC]=cqw cols, [:, HC:2HC]=cw cols, [:, 2HC]=bias (col j is h=j*128+p)
    wv = np.concatenate(
        [
            cqw.reshape(HC, P).T,
            cw.reshape(HC, P).T,
            np.full((P, 1), bs, np.float32),
        ],
        axis=1,
    ).astype(np.float32)
    wv = np.ascontiguousarray(wv)
    qwr = qw.reshape(HC, P).T.astype(bf16)

    in_maps = []
    for i in range(N_CORES):
        sl = slice(i * BPC, (i + 1) * BPC)
        ctx_i = context[sl]
        qry_i = query[sl]
        # ctx: [b, c, h] -> [b, p, t, h] with c = t*128+p
        ctx_s = np.ascontiguousarray(
            ctx_i.reshape(BPC, CT, P, H).transpose(0, 2, 1, 3).reshape(BPC, P, CT * H)
        ).astype(bf16)
        # ctxT: [b, h, c] -> [b, p, j, c] with h = j*128+p
        ctxT_s = np.ascontiguousarray(
            ctx_i.transpose(0, 2, 1)
            .reshape(BPC, HC, P, C)
            .transpose(0, 2, 1, 3)
            .reshape(BPC, P, HC * C)
        ).astype(bf16)
        # qry^T: [b, h, q] -> [p, j, b, q]
        qt_s = (
            qry_i.transpose(0, 2, 1)
            .reshape(BPC, HC, P, Q)
            .transpose(2, 1, 0, 3)
            .reshape(P, HC * BPC * Q)
        ).astype(bf16)
        # qry: [b, q, h] -> [q, b, h]
        qa_s = qry_i.transpose(1, 0, 2).reshape(P, BPC * H).astype(bf16)
        qb = np.ascontiguousarray(np.concatenate([qt_s, qa_s, qwr], axis=1))
        in_maps.append({"ctx": ctx_s, "ctxT": ctxT_s, "qb": qb, "wv": wv})
    return in_maps


def kernel(context, query, c_mask, q_mask, c_weight, q_weight, cq_weight, bias):
    from concourse import bass_utils

    nc = _get_nc()
    in_maps = make_in_maps(context, query, c_weight, q_weight, cq_weight, bias)
    res = bass_utils.run_bass_kernel_spmd(nc, in_maps, core_ids=list(range(N_CORES)))

    context = np.asarray(context, dtype=np.float32)
    full = np.empty((B, C, 4 * H), dtype=np.float32)
    full[:, :, 0:H] = context
    for i in range(N_CORES):
        # device out: [b, p, t, 3h] -> [b, (t p), 3h]
        o = res.results[i]["out"].reshape(BPC, P, CT, 3 * H).transpose(0, 2, 1, 3)
        full[i * BPC : (i + 1) * BPC, :, H:] = o.reshape(BPC, C, 3 * H).astype(
            np.float32
        )
    return full


# revision 7
# speedup vs baseline: 1.6439x; 1.0168x over previous
"""Trainium2 Bass kernel for ContextQueryAttention (BiDAF-style trilinear
attention). Data-parallel over batch across 8 NeuronCores (4 batches/core).

Per batch (c=1024 context rows, q=128 query rows, h=256 hidden):
  S[c,q]   = ctx@cw + (qry@qw)^T + (ctx*cqw)@qry^T + bias
  S_bar    = softmax_c(S); S_bar_bar = softmax_q(S)
  A        = S @ qry
  B        = S_bar @ (S_bar_bar^T @ ctx)
  out      = concat([ctx, A, ctx*A, ctx*B], -1)

v4: all-bf16 I/O + software-pipelined batches.
  - ctx@cw (s0) folds into the S^T contraction (lhsT = qry^T*cqw + cw);
    s1+bias rides one K=1 rank-1 matmul per 512-col chunk.
  - One exp pass (fused row-sums) serves both softmaxes: 1/Zc folds into
    the transposed exp tiles, 1/Zq into T.
  - PE stream per iteration: S^T(b), B(b-1), tr(b), A(b), T(b) — the
    deferred B-phase keeps the PE continuously fed (p-state ramp) and
    gives the zc/rc/e-scale chain time to complete off-engine.
  - Evac split: exp/straw0/es-odd/A-copy on ACT; straw1/ctxB/zc/es-even/
    ts/ctxA01 on DVE; ctxA23 on Pool (SBUF-only engine).
  - Outputs bf16, ctx passthrough channel assembled host-side.
"""

import numpy as np

B, C, Q, H = 32, 1024, 128, 256
N_CORES = 8
BPC = B // N_CORES  # batches per core
P = 128
HC = H // P  # h chunks of 128
CT = C // P  # c tiles of 128
CCH = 512  # S^T free-dim chunk (1 PSUM bank of fp32)
NCC = C // CCH

_NC_CACHE = {}


def _build_kernel():
    import concourse.bacc as bacc
    import concourse.tile as tile
    from concourse import mybir
    from concourse.masks import make_identity

    f32 = mybir.dt.float32
    bf16 = mybir.dt.bfloat16
    AF = mybir.ActivationFunctionType
    AX = mybir.AxisListType
    ALU = mybir.AluOpType

    nc = bacc.Bacc(trn_type="TRN2", target_bir_lowering=False, debug=False)
    ctx_d = nc.dram_tensor("ctx", [BPC, P, CT * H], bf16, kind="ExternalInput").ap()
    ctxT_d = nc.dram_tensor("ctxT", [BPC, P, HC * C], bf16, kind="ExternalInput").ap()
    # packed bf16 consts: [0:1024]=qry^T, [1024:2048]=qry, [2048:2050]=qw cols
    qb_d = nc.dram_tensor("qb", [P, 2 * HC * BPC * Q + HC], bf16, kind="ExternalInput").ap()
    # packed f32 consts: [0:HC]=cqw cols, [HC:2HC]=cw cols, [2HC]=bias
    wv_d = nc.dram_tensor("wv", [P, 2 * HC + 1], f32, kind="ExternalInput").ap()
    out_d = nc.dram_tensor("out", [BPC, P, CT * 3 * H], bf16, kind="ExternalOutput").ap()

    from contextlib import ExitStack

    with tile.TileContext(nc) as tc, ExitStack() as es:
        consts = es.enter_context(tc.tile_pool(name="consts", bufs=1))
        p_et = es.enter_context(tc.tile_pool(name="p_et", bufs=2))
        p_sr = es.enter_context(tc.tile_pool(name="p_sr", bufs=2))
        p_esb = es.enter_context(tc.tile_pool(name="p_esb", bufs=2))
        p_out = es.enter_context(tc.tile_pool(name="p_out", bufs=2))
        p_vec = es.enter_context(tc.tile_pool(name="p_vec", bufs=2))
        pp_st = es.enter_context(tc.tile_pool(name="pp_st", bufs=2, space="PSUM"))
        pp_tr = es.enter_context(tc.tile_pool(name="pp_tr", bufs=1, space="PSUM"))
        pp_t = es.enter_context(tc.tile_pool(name="pp_t", bufs=1, space="PSUM"))
        pp_ab = es.enter_context(tc.tile_pool(name="pp_ab", bufs=4, space="PSUM"))

        # ---- const DMAs (2 packed transfers) ----
        wv = consts.tile([P, 2 * HC + 1], f32)
        nc.sync.dma_start(out=wv, in_=wv_d)
        qb = consts.tile([P, 2 * HC * BPC * Q + HC], bf16)
        nc.sync.dma_start(out=qb, in_=qb_d)
        qt_all = qb[:, 0 : HC * BPC * Q].rearrange("p (j bq) -> p j bq", j=HC)
        qa_all = qb[:, HC * BPC * Q : 2 * HC * BPC * Q].rearrange(
            "p (b h) -> p b h", b=BPC
        )
        qwr = qb[:, 2 * HC * BPC * Q :]
        bias_sb = wv[0:1, 2 * HC : 2 * HC + 1]

        identity = consts.tile([P, P], bf16)
        make_identity(nc, identity)
        ones_c = consts.tile([1, CCH], bf16)
        nc.vector.memset(ones_c, 1.0)

        # ---- big input DMAs for first two batches ----
        ctxT_all = consts.tile([P, BPC, HC, C], bf16)
        ctx_all = consts.tile([P, BPC, CT, H], bf16)
        for b in range(2):
            nc.sync.dma_start(
                out=ctxT_all[:, b].rearrange("p j c -> p (j c)"), in_=ctxT_d[b]
            )
            nc.sync.dma_start(
                out=ctx_all[:, b].rearrange("p t h -> p (t h)"), in_=ctx_d[b]
            )

        # ---- preamble: qt_cq = qry^T*cqw + cw (folds s0) on Pool; s1 rows ----
        qt_cq = consts.tile([P, HC, BPC * Q], bf16)
        for j in range(HC):
            nc.gpsimd.tensor_scalar(
                qt_cq[:, j],
                qt_all[:, j],
                wv[:, j : j + 1],
                wv[:, HC + j : HC + j + 1],
                ALU.mult,
                ALU.add,
            )
        s1p = pp_st.tile([1, BPC * Q], f32, tag="stp")
        for j in range(HC):
            nc.tensor.matmul(
                s1p,
                lhsT=qwr[:, j : j + 1],
                rhs=qt_all[:, j],
                start=(j == 0),
                stop=(j == HC - 1),
            )
        s1_rows = consts.tile([1, BPC * Q], bf16)
        nc.scalar.activation(s1_rows, s1p, AF.Identity, bias=bias_sb, scale=1.0)

        # cross-iteration state for the deferred B-phase of batch b-1
        prev = None  # (b, e_t, ts, out_t)

        def emit_b_phase(state):
            bp, e_tp, tsp, out_tp = state
            for p2 in range(CT // 2):
                t0 = 2 * p2
                pb = pp_ab.tile([P, 2, H], f32, tag="ab", name=f"pb{bp}{p2}")
                for k in range(2):
                    nc.tensor.matmul(
                        pb[:, k, :],
                        lhsT=e_tp[:, (t0 + k) * P : (t0 + k + 1) * P],
                        rhs=tsp,
                        start=True,
                        stop=True,
                    )
                nc.vector.tensor_mul(
                    out_tp[:, t0 : t0 + 2, 2 * H : 3 * H],
                    ctx_all[:, bp, t0 : t0 + 2, :],
                    pb,
                )
            # stores in halves so the first can drain early
            nc.sync.dma_start(
                out=out_d[bp, :, 0 : 4 * 3 * H],
                in_=out_tp[:, 0:4, :].rearrange("p t h3 -> p (t h3)"),
            )
            nc.sync.dma_start(
                out=out_d[bp, :, 4 * 3 * H :],
                in_=out_tp[:, 4:CT, :].rearrange("p t h3 -> p (t h3)"),
            )

        for b in range(BPC):
            if b + 2 < BPC:
                nc.sync.dma_start(
                    out=ctxT_all[:, b + 2].rearrange("p j c -> p (j c)"),
                    in_=ctxT_d[b + 2],
                )
                nc.sync.dma_start(
                    out=ctx_all[:, b + 2].rearrange("p t h -> p (t h)"),
                    in_=ctx_d[b + 2],
                )

            bq = slice(b * Q, (b + 1) * Q)

            # ---- S^T [q, c] (incl s0 via qt_cq, s1+bias via rider); exp ----
            e_t = p_et.tile([P, C], bf16, tag="e_t")
            st_raw = p_sr.tile([P, C], bf16, tag="st_raw")
            rsum = p_vec.tile([P, NCC], f32, tag="rsum")
            stps = []
            for cc in range(NCC):
                sl = slice(cc * CCH, (cc + 1) * CCH)
                stp = pp_st.tile([P, CCH], f32, tag="stp")
                for j in range(HC):
                    nc.tensor.matmul(
                        stp,
                        lhsT=qt_cq[:, j, bq],
                        rhs=ctxT_all[:, b, j, sl],
                        start=(j == 0),
                        stop=False,
                    )
                nc.tensor.matmul(
                    stp, lhsT=s1_rows[0:1, bq], rhs=ones_c, start=False, stop=True
                )
                nc.scalar.activation(
                    e_t[:, sl], stp, AF.Exp, accum_out=rsum[:, cc : cc + 1]
                )
                stps.append(stp)
            # raw S^T to SBUF for the A matmuls (ACT c0, DVE c1)
            nc.scalar.copy(st_raw[:, 0:CCH], stps[0])
            nc.vector.tensor_copy(st_raw[:, CCH:C], stps[1])

            # ---- deferred B-phase of batch b-1 fills the PE here ----
            if prev is not None:
                emit_b_phase(prev)

            # ---- E-transposes ----
            tr8 = pp_tr.tile([P, CT, P], bf16, tag="tr8")
            for t in range(CT):
                nc.tensor.transpose(tr8[:, t, :], e_t[:, t * P : (t + 1) * P], identity)

            # softmax_c denominators: rq[q] = 1 / sum_c exp
            zq = p_vec.tile([P, 1], f32, tag="zq")
            nc.vector.reduce_sum(zq, rsum, axis=AX.X)
            rq = p_vec.tile([P, 1], f32, tag="rq")
            nc.vector.reciprocal(rq, zq)

            # per-tile softmax_q normalizers (halves); e-scale DVE-even/ACT-odd
            e_sb = p_esb.tile([P, CT, P], bf16, tag="e_sb")
            zc8 = p_vec.tile([P, CT], f32, tag="zc8")
            rc8 = p_vec.tile([P, CT], f32, tag="rc8")
            h1 = CT // 2
            nc.vector.reduce_sum(zc8[:, 0:h1], tr8[:, 0:h1, :], axis=AX.X)
            nc.vector.reciprocal(rc8[:, 0:h1], zc8[:, 0:h1])
            nc.vector.reduce_sum(zc8[:, h1:CT], tr8[:, h1:CT, :], axis=AX.X)
            nc.vector.reciprocal(rc8[:, h1:CT], zc8[:, h1:CT])
            for t in range(CT):
                if t % 2 == 0:
                    nc.vector.tensor_scalar_mul(
                        e_sb[:, t, :], tr8[:, t, :], rc8[:, t : t + 1]
                    )
                else:
                    nc.scalar.mul(e_sb[:, t, :], tr8[:, t, :], rc8[:, t : t + 1])

            out_t = p_out.tile([P, CT, 3 * H], bf16, tag="out_t")

            # ---- A per c-tile pair; copy on ACT, ctx*A on DVE(bf16)/Pool ----
            for p2 in range(CT // 2):
                t0 = 2 * p2
                pa = pp_ab.tile([P, 2, H], f32, tag="ab", name=f"pa{b}{p2}")
                for k in range(2):
                    nc.tensor.matmul(
                        pa[:, k, :],
                        lhsT=st_raw[:, (t0 + k) * P : (t0 + k + 1) * P],
                        rhs=qa_all[:, b, :],
                        start=True,
                        stop=True,
                    )
                nc.scalar.copy(out_t[:, t0 : t0 + 2, 0:H], pa)
                if p2 >= 2:
                    nc.gpsimd.tensor_mul(
                        out_t[:, t0 : t0 + 2, H : 2 * H],
                        ctx_all[:, b, t0 : t0 + 2, :],
                        out_t[:, t0 : t0 + 2, 0:H],
                    )

            # ---- T = S_bar_bar^T @ ctx as one tight accumulation group ----
            t_acc = pp_t.tile([P, H], f32, tag="t_acc")
            for t in range(CT):
                nc.tensor.matmul(
                    t_acc,
                    lhsT=e_sb[:, t, :],
                    rhs=ctx_all[:, b, t, :],
                    start=(t == 0),
                    stop=(t == CT - 1),
                )
            ts = p_vec.tile([P, H], bf16, tag="ts")
            nc.vector.tensor_scalar_mul(ts, t_acc, rq)

            # ctx*A for pairs 0,1 on DVE at stream end (all-bf16, after ts)
            for p2 in range(2):
                t0 = 2 * p2
                nc.vector.tensor_mul(
                    out_t[:, t0 : t0 + 2, H : 2 * H],
                    ctx_all[:, b, t0 : t0 + 2, :],
                    out_t[:, t0 : t0 + 2, 0:H],
                )

            prev = (b, e_t, ts, out_t)

        emit_b_phase(prev)

    nc.compile()
    return nc


def _get_nc():
    if "nc" not in _NC_CACHE:
        _NC_CACHE["nc"] = _build_kernel()
    return _NC_CACHE["nc"]


def make_in_maps(context, query, c_weight, q_weight, cq_weight, bias):
    import ml_dtypes

    bf16 = ml_dtypes.bfloat16
    context = np.ascontiguousarray(np.asarray(context, dtype=np.float32))
    query = np.asarray(query, dtype=np.float32)
    cw = np.asarray(c_weight, dtype=np.float32).reshape(H)
    qw = np.asarray(q_weight, dtype=np.float32).reshape(H)
    cqw = np.asarray(cq_weight, dtype=np.float32).reshape(H)
    bs = float(np.asarray(bias, dtype=np.float32).reshape(1)[0])

    # [:, 0:HC]=cqw cols, [:, HC:2HC]=cw cols, [:, 2HC]=bias (col j is h=j*128+p)
    wv = np.concatenate(
        [
            cqw.reshape(HC, P).T,
            cw.reshape(HC, P).T,
            np.full((P, 1), bs, np.float32),
        ],
        axis=1,
    ).astype(np.float32)
    wv = np.ascontiguousarray(wv)
    qwr = qw.reshape(HC, P).T.astype(bf16)

    in_maps = []
    for i in range(N_CORES):
        sl = slice(i * BPC, (i + 1) * BPC)
        ctx_i = context[sl]
        qry_i = query[sl]
        # ctx: [b, c, h] -> [b, p, t, h] with c = t*128+p
        ctx_s = np.ascontiguousarray(
            ctx_i.reshape(BPC, CT, P, H).transpose(0, 2, 1, 3).reshape(BPC, P, CT * H)
        ).astype(bf16)
        # ctxT: [b, h, c] -> [b, p, j, c] with h = j*128+p
        ctxT_s = np.ascontiguousarray(
            ctx_i.transpose(0, 2, 1)
            .reshape(BPC, HC, P, C)
            .transpose(0, 2, 1, 3)
            .reshape(BPC, P, HC * C)
        ).astype(bf16)
        # qry^T: [b, h, q] -> [p, j, b, q]
        qt_s = (
            qry_i.transpose(0, 2, 1)
            .reshape(BPC, HC, P, Q)
            .transpose(2, 1, 0, 3)
            .reshape(P, HC * BPC * Q)
        ).astype(bf16)
        # qry: [b, q, h] -> [q, b, h]
        qa_s = qry_i.transpose(1, 0, 2).reshape(P, BPC * H).astype(bf16)
        qb = np.ascontiguousarray(np.concatenate([qt_s, qa_s, qwr], axis=1))
        in_maps.append({"ctx": ctx_s, "ctxT": ctxT_s, "qb": qb, "wv": wv})
    return in_maps


def kernel(context, query, c_mask, q_mask, c_weight, q_weight, cq_weight, bias):
    from concourse import bass_utils

    nc = _get_nc()
    in_maps = make_in_maps(context, query, c_weight, q_weight, cq_weight, bias)
    res = bass_utils.run_bass_kernel_spmd(nc, in_maps, core_ids=list(range(N_CORES)))

    context = np.asarray(context, dtype=np.float32)
    full = np.empty((B, C, 4 * H), dtype=np.float32)
    full[:, :, 0:H] = context
    for i in range(N_CORES):
        # device out: [b, p, t, 3h] -> [b, (t p), 3h]
        o = res.results[i]["out"].reshape(BPC, P, CT, 3 * H).transpose(0, 2, 1, 3)
        full[i * BPC : (i + 1) * BPC, :, H:] = o.reshape(BPC, C, 3 * H).astype(
            np.float32
        )
    return full


# revision 8
# speedup vs baseline: 1.7252x; 1.0495x over previous
"""Trainium2 Bass kernel for ContextQueryAttention (BiDAF-style trilinear
attention). Data-parallel over batch across 8 NeuronCores (4 batches/core).

Per batch (c=1024 context rows, q=128 query rows, h=256 hidden):
  S[c,q]   = ctx@cw + (qry@qw)^T + (ctx*cqw)@qry^T + bias
  S_bar    = softmax_c(S); S_bar_bar = softmax_q(S)
  A        = S @ qry
  B        = S_bar @ (S_bar_bar^T @ ctx)
  out      = concat([ctx, A, ctx*A, ctx*B], -1)

v5: all-bf16 I/O + 2-deep software pipelining.
  - ctx@cw (s0) folds into the S^T contraction (lhsT = qry^T*cqw + cw);
    s1+bias rides one K=1 rank-1 matmul per 512-col chunk.
  - One exp pass (fused row-sums) serves both softmaxes: 1/Zc folds into
    the transposed exp tiles, 1/Zq into T.
  - T(b) and B(b) matmuls are deferred one iteration: the per-tile
    zc -> 1/zc -> e-scale chain of batch b gets a full cycle of slack, so
    every PE phase in the steady state has pre-satisfied inputs:
      PE/iter b: S^T(b), T(b-1), A(b), transposes(b), B(b-1).
  - Evac split: exp/A-copy/es-odd on ACT; straws/ts/ctxB/zc/es-even on
    DVE; zq/ctx*A on Pool (SBUF-only engine; ctx*A reads the copied
    SBUF A-channel).
  - Outputs bf16, ctx passthrough channel assembled host-side.
"""

import numpy as np

B, C, Q, H = 32, 1024, 128, 256
N_CORES = 8
BPC = B // N_CORES  # batches per core
P = 128
HC = H // P  # h chunks of 128
CT = C // P  # c tiles of 128
CCH = 512  # S^T free-dim chunk (1 PSUM bank of fp32)
NCC = C // CCH

_NC_CACHE = {}


def _build_kernel():
    import concourse.bacc as bacc
    import concourse.tile as tile
    from concourse import mybir
    from concourse.masks import make_identity

    f32 = mybir.dt.float32
    bf16 = mybir.dt.bfloat16
    AF = mybir.ActivationFunctionType
    AX = mybir.AxisListType
    ALU = mybir.AluOpType

    nc = bacc.Bacc(trn_type="TRN2", target_bir_lowering=False, debug=False)
    ctx_d = nc.dram_tensor("ctx", [BPC, P, CT * H], bf16, kind="ExternalInput").ap()
    ctxT_d = nc.dram_tensor("ctxT", [BPC, P, HC * C], bf16, kind="ExternalInput").ap()
    # packed bf16 consts: [0:1024]=qry^T, [1024:2048]=qry, [2048:2050]=qw cols
    qb_d = nc.dram_tensor("qb", [P, 2 * HC * BPC * Q + HC], bf16, kind="ExternalInput").ap()
    # packed f32 consts: [0:HC]=cqw cols, [HC:2HC]=cw cols, [2HC]=bias
    wv_d = nc.dram_tensor("wv", [P, 2 * HC + 1], f32, kind="ExternalInput").ap()
    out_d = nc.dram_tensor("out", [BPC, P, CT * 3 * H], bf16, kind="ExternalOutput").ap()

    from contextlib import ExitStack

    with tile.TileContext(nc) as tc, ExitStack() as es:
        consts = es.enter_context(tc.tile_pool(name="consts", bufs=1))
        p_et = es.enter_context(tc.tile_pool(name="p_et", bufs=2))
        p_sr = es.enter_context(tc.tile_pool(name="p_sr", bufs=2))
        p_esb = es.enter_context(tc.tile_pool(name="p_esb", bufs=2))
        p_out = es.enter_context(tc.tile_pool(name="p_out", bufs=2))
        p_vec = es.enter_context(tc.tile_pool(name="p_vec", bufs=2))
        pp_st = es.enter_context(tc.tile_pool(name="pp_st", bufs=2, space="PSUM"))
        pp_tr = es.enter_context(tc.tile_pool(name="pp_tr", bufs=1, space="PSUM"))
        pp_t = es.enter_context(tc.tile_pool(name="pp_t", bufs=1, space="PSUM"))
        pp_a = es.enter_context(tc.tile_pool(name="pp_a", bufs=2, space="PSUM"))
        pp_b = es.enter_context(tc.tile_pool(name="pp_b", bufs=2, space="PSUM"))

        # ---- const DMAs (2 packed transfers) ----
        wv = consts.tile([P, 2 * HC + 1], f32)
        nc.sync.dma_start(out=wv, in_=wv_d)
        qb = consts.tile([P, 2 * HC * BPC * Q + HC], bf16)
        nc.sync.dma_start(out=qb, in_=qb_d)
        qt_all = qb[:, 0 : HC * BPC * Q].rearrange("p (j bq) -> p j bq", j=HC)
        qa_all = qb[:, HC * BPC * Q : 2 * HC * BPC * Q].rearrange(
            "p (b h) -> p b h", b=BPC
        )
        qwr = qb[:, 2 * HC * BPC * Q :]
        bias_sb = wv[0:1, 2 * HC : 2 * HC + 1]

        identity = consts.tile([P, P], bf16)
        make_identity(nc, identity)
        ones_c = consts.tile([1, CCH], bf16)
        nc.vector.memset(ones_c, 1.0)

        # ---- big input DMAs for first two batches ----
        ctxT_all = consts.tile([P, BPC, HC, C], bf16)
        ctx_all = consts.tile([P, BPC, CT, H], bf16)
        for b in range(2):
            nc.sync.dma_start(
                out=ctxT_all[:, b].rearrange("p j c -> p (j c)"), in_=ctxT_d[b]
            )
            nc.sync.dma_start(
                out=ctx_all[:, b].rearrange("p t h -> p (t h)"), in_=ctx_d[b]
            )

        # ---- preamble: qt_cq = qry^T*cqw + cw (folds s0) on Pool; s1 rows ----
        qt_cq = consts.tile([P, HC, BPC * Q], bf16)
        for j in range(HC):
            nc.gpsimd.tensor_scalar(
                qt_cq[:, j],
                qt_all[:, j],
                wv[:, j : j + 1],
                wv[:, HC + j : HC + j + 1],
                ALU.mult,
                ALU.add,
            )
        s1p = pp_st.tile([1, BPC * Q], f32, tag="stp")
        for j in range(HC):
            nc.tensor.matmul(
                s1p,
                lhsT=qwr[:, j : j + 1],
                rhs=qt_all[:, j],
                start=(j == 0),
                stop=(j == HC - 1),
            )
        s1_rows = consts.tile([1, BPC * Q], bf16)
        nc.scalar.activation(s1_rows, s1p, AF.Identity, bias=bias_sb, scale=1.0)

        # cross-iteration state of batch b-1: (b, e_t, e_sb, rq, out_t)
        prev = None

        def emit_t_phase(state):
            """T(b-1) = S_bar_bar^T @ ctx; ts = T * rq.  Returns ts."""
            bp, e_tp, e_sbp, rqp, _ = state
            t_acc = pp_t.tile([P, H], f32, tag="t_acc", name=f"tacc{bp}")
            for t in range(CT):
                nc.tensor.matmul(
                    t_acc,
                    lhsT=e_sbp[:, t, :],
                    rhs=ctx_all[:, bp, t, :],
                    start=(t == 0),
                    stop=(t == CT - 1),
                )
            ts = p_vec.tile([P, H], bf16, tag="ts", name=f"ts{bp}")
            nc.vector.tensor_scalar_mul(ts, t_acc, rqp)
            return ts

        def emit_b_phase(state, ts):
            """B(b-1) pairs + ctx*B on DVE, then store batch b-1."""
            bp, e_tp, _, _, out_tp = state
            for p2 in range(CT // 2):
                t0 = 2 * p2
                pb = pp_b.tile([P, 2, H], f32, tag="pb", name=f"pb{bp}{p2}")
                for k in range(2):
                    nc.tensor.matmul(
                        pb[:, k, :],
                        lhsT=e_tp[:, (t0 + k) * P : (t0 + k + 1) * P],
                        rhs=ts,
                        start=True,
                        stop=True,
                    )
                nc.vector.tensor_mul(
                    out_tp[:, t0 : t0 + 2, 2 * H : 3 * H],
                    ctx_all[:, bp, t0 : t0 + 2, :],
                    pb,
                )
            nc.sync.dma_start(
                out=out_d[bp, :, 0 : 4 * 3 * H],
                in_=out_tp[:, 0:4, :].rearrange("p t h3 -> p (t h3)"),
            )
            nc.sync.dma_start(
                out=out_d[bp, :, 4 * 3 * H :],
                in_=out_tp[:, 4:CT, :].rearrange("p t h3 -> p (t h3)"),
            )

        for b in range(BPC):
            if b + 2 < BPC:
                nc.sync.dma_start(
                    out=ctxT_all[:, b + 2].rearrange("p j c -> p (j c)"),
                    in_=ctxT_d[b + 2],
                )
                nc.sync.dma_start(
                    out=ctx_all[:, b + 2].rearrange("p t h -> p (t h)"),
                    in_=ctx_d[b + 2],
                )

            bq = slice(b * Q, (b + 1) * Q)

            # ---- S^T [q, c] (incl s0 via qt_cq, s1+bias via rider); exp ----
            e_t = p_et.tile([P, C], bf16, tag="e_t")
            st_raw = p_sr.tile([P, C], bf16, tag="st_raw")
            rsum = p_vec.tile([P, NCC], f32, tag="rsum")
            for cc in range(NCC):
                sl = slice(cc * CCH, (cc + 1) * CCH)
                stp = pp_st.tile([P, CCH], f32, tag="stp")
                for j in range(HC):
                    nc.tensor.matmul(
                        stp,
                        lhsT=qt_cq[:, j, bq],
                        rhs=ctxT_all[:, b, j, sl],
                        start=(j == 0),
                        stop=False,
                    )
                nc.tensor.matmul(
                    stp, lhsT=s1_rows[0:1, bq], rhs=ones_c, start=False, stop=True
                )
                nc.scalar.activation(
                    e_t[:, sl], stp, AF.Exp, accum_out=rsum[:, cc : cc + 1]
                )
                nc.vector.tensor_copy(st_raw[:, sl], stp)

            # softmax_c denominators (zq on Pool, reciprocal on DVE)
            zq = p_vec.tile([P, 1], f32, tag="zq")
            nc.gpsimd.tensor_add(zq, rsum[:, 0:1], rsum[:, 1:2])
            rq = p_vec.tile([P, 1], f32, tag="rq")
            nc.vector.reciprocal(rq, zq)

            # ---- deferred T-phase of batch b-1 (its e_sb is long ready) ----
            ts_prev = emit_t_phase(prev) if prev is not None else None

            out_t = p_out.tile([P, CT, 3 * H], bf16, tag="out_t")

            # ---- A per c-tile pair; copy on ACT, ctx*A on Pool (SBUF) ----
            for p2 in range(CT // 2):
                t0 = 2 * p2
                pa = pp_a.tile([P, 2, H], f32, tag="pa", name=f"pa{b}{p2}")
                for k in range(2):
                    nc.tensor.matmul(
                        pa[:, k, :],
                        lhsT=st_raw[:, (t0 + k) * P : (t0 + k + 1) * P],
                        rhs=qa_all[:, b, :],
                        start=True,
                        stop=True,
                    )
                nc.scalar.copy(out_t[:, t0 : t0 + 2, 0:H], pa)
                nc.gpsimd.tensor_mul(
                    out_t[:, t0 : t0 + 2, H : 2 * H],
                    ctx_all[:, b, t0 : t0 + 2, :],
                    out_t[:, t0 : t0 + 2, 0:H],
                )

            # ---- E-transposes; zc/rc/e-scale (consumed only next iter) ----
            tr8 = pp_tr.tile([P, CT, P], bf16, tag="tr8")
            for t in range(CT):
                nc.tensor.transpose(tr8[:, t, :], e_t[:, t * P : (t + 1) * P], identity)
            e_sb = p_esb.tile([P, CT, P], bf16, tag="e_sb")
            zc8 = p_vec.tile([P, CT], f32, tag="zc8")
            rc8 = p_vec.tile([P, CT], f32, tag="rc8")
            nc.vector.reduce_sum(zc8, tr8, axis=AX.X)
            nc.vector.reciprocal(rc8, zc8)
            for t in range(CT):
                if t % 2 == 0:
                    nc.vector.tensor_scalar_mul(
                        e_sb[:, t, :], tr8[:, t, :], rc8[:, t : t + 1]
                    )
                else:
                    nc.scalar.mul(e_sb[:, t, :], tr8[:, t, :], rc8[:, t : t + 1])

            # ---- deferred B-phase + store of batch b-1 ----
            if prev is not None:
                emit_b_phase(prev, ts_prev)

            prev = (b, e_t, e_sb, rq, out_t)

        # epilogue: T/B/store for the last batch
        ts_last = emit_t_phase(prev)
        emit_b_phase(prev, ts_last)

    nc.compile()
    return nc


def _get_nc():
    if "nc" not in _NC_CACHE:
        _NC_CACHE["nc"] = _build_kernel()
    return _NC_CACHE["nc"]


def make_in_maps(context, query, c_weight, q_weight, cq_weight, bias):
    import ml_dtypes

    bf16 = ml_dtypes.bfloat16
    context = np.ascontiguousarray(np.asarray(context, dtype=np.float32))
    query = np.asarray(query, dtype=np.float32)
    cw = np.asarray(c_weight, dtype=np.float32).reshape(H)
    qw = np.asarray(q_weight, dtype=np.float32).reshape(H)
    cqw = np.asarray(cq_weight, dtype=np.float32).reshape(H)
    bs = float(np.asarray(bias, dtype=np.float32).reshape(1)[0])

    # [:, 0:HC]=cqw cols, [:, HC:2HC]=cw cols, [:, 2HC]=bias (col j is h=j*128+p)
    wv = np.concatenate(
        [
            cqw.reshape(HC, P).T,
            cw.reshape(HC, P).T,
            np.full((P, 1), bs, np.float32),
        ],
        axis=1,
    ).astype(np.float32)
    wv = np.ascontiguousarray(wv)
    qwr = qw.reshape(HC, P).T.astype(bf16)

    in_maps = []
    for i in range(N_CORES):
        sl = slice(i * BPC, (i + 1) * BPC)
        ctx_i = context[sl]
        qry_i = query[sl]
        # ctx: [b, c, h] -> [b, p, t, h] with c = t*128+p
        ctx_s = np.ascontiguousarray(
            ctx_i.reshape(BPC, CT, P, H).transpose(0, 2, 1, 3).reshape(BPC, P, CT * H)
        ).astype(bf16)
        # ctxT: [b, h, c] -> [b, p, j, c] with h = j*128+p
        ctxT_s = np.ascontiguousarray(
            ctx_i.transpose(0, 2, 1)
            .reshape(BPC, HC, P, C)
            .transpose(0, 2, 1, 3)
            .reshape(BPC, P, HC * C)
        ).astype(bf16)
        # qry^T: [b, h, q] -> [p, j, b, q]
        qt_s = (
            qry_i.transpose(0, 2, 1)
            .reshape(BPC, HC, P, Q)
            .transpose(2, 1, 0, 3)
            .reshape(P, HC * BPC * Q)
        ).astype(bf16)
        # qry: [b, q, h] -> [q, b, h]
        qa_s = qry_i.transpose(1, 0, 2).reshape(P, BPC * H).astype(bf16)
        qb = np.ascontiguousarray(np.concatenate([qt_s, qa_s, qwr], axis=1))
        in_maps.append({"ctx": ctx_s, "ctxT": ctxT_s, "qb": qb, "wv": wv})
    return in_maps


def kernel(context, query, c_mask, q_mask, c_weight, q_weight, cq_weight, bias):
    from concourse import bass_utils

    nc = _get_nc()
    in_maps = make_in_maps(context, query, c_weight, q_weight, cq_weight, bias)
    res = bass_utils.run_bass_kernel_spmd(nc, in_maps, core_ids=list(range(N_CORES)))

    context = np.asarray(context, dtype=np.float32)
    full = np.empty((B, C, 4 * H), dtype=np.float32)
    full[:, :, 0:H] = context
    for i in range(N_CORES):
        # device out: [b, p, t, 3h] -> [b, (t p), 3h]
        o = res.results[i]["out"].reshape(BPC, P, CT, 3 * H).transpose(0, 2, 1, 3)
        full[i * BPC : (i + 1) * BPC, :, H:] = o.reshape(BPC, C, 3 * H).astype(
            np.float32
        )
    return full
